# revision 1
# baseline (speedup 1.0000x reference)
"""Trainium2 Bass kernel for nn_BlocksparseFixedSelfAttention.

Reference computation (B=4, T=2048, EMB=512, KBLK=64):
    Kt = x @ Wk.T + bk ; Qt = x @ Wq.T + bq ; Vt = x @ Wv.T + bv
    head1: block-causal local attention inside each 64-token block
           (row j attends cols [block_start(j) .. j], S = K Q^T)
    head2: row r attends every block start c = 64*i with c <= r
    out = concat(h1, h2) @ Wu.T + bu

Sharding: data-parallel over (batch, T-half) -> 8 shards, one per core.
Each core gets its 1024 own token rows of x plus the 32 block-start
rows (needed for head2's Q/V at block starts), replicated weights, and
produces its [1024, 512] slice of the output. x is shipped
feature-major (x^T) so the contraction dim lands on SBUF partitions.

Device dataflow (per core), all matmuls in fp32r (fast fp32 mode):
    K^T,Q^T = W^T.T @ x^T   (N=512 moving)      [e, t]
    V       = x^T.T @ Wv^T  (N=512 moving)      [t, e] natural
    S1^T[c, r] = Q K^T per 128-token tile, masked to in-block pattern
    h1^T[e, r] = V_tile.T @ S1t
    S2^T[i, r] = Qs K^T  (i = 32 block starts), masked by i <= r//64
    h2^T[e, r] = Vs.T @ S2t
    out[t, d] = hcat^T.T @ Wu^T + bu

Biases: bk/bq are per-partition scalars fused into the PSUM->SBUF copy;
bv/bu are broadcast across partitions once via a rank-1 ones matmul and
added during the copy. Measured on HW: ~80us exec, rel err 3.6e-4.
"""

import os
import sys

import numpy as np

for _p in ("/opt/trn_rl_repo",):
    if _p not in sys.path and os.path.isdir(_p):
        sys.path.append(_p)

from concourse import bass, bacc, mybir
from concourse import tile
from concourse.bass_utils import run_bass_kernel_spmd

T = 2048
KBLK = 64
EMB = 512
B = 4
NCORES = 8
HALF = T // 2            # tokens owned per core
NSTART = T // KBLK       # 32 block starts
TOT = HALF + NSTART      # own tokens + appended block-start tokens
F32 = mybir.dt.float32
F32R = mybir.dt.float32r

# Score/AV matmuls in bf16 (1 cyc/row on PE instead of 4 for fp32r N<256).
BF16_ATTN = False
BF16 = mybir.dt.bfloat16


def build_program():
    nc = bacc.Bacc("TRN2", target_bir_lowering=False, debug=False)

    xt_d = nc.declare_dram_parameter("xt", [EMB, TOT], F32, False)
    wkt_d = nc.declare_dram_parameter("wkt", [EMB, EMB], F32, False)
    wqt_d = nc.declare_dram_parameter("wqt", [EMB, EMB], F32, False)
    wvt_d = nc.declare_dram_parameter("wvt", [EMB, EMB], F32, False)
    wut_d = nc.declare_dram_parameter("wut", [2 * EMB, EMB], F32, False)
    bk_d = nc.declare_dram_parameter("bkc", [128, EMB // 128], F32, False)
    bq_d = nc.declare_dram_parameter("bqc", [128, EMB // 128], F32, False)
    bv_d = nc.declare_dram_parameter("bvr", [1, EMB], F32, False)
    bu_d = nc.declare_dram_parameter("bur", [1, EMB], F32, False)
    m1_d = nc.declare_dram_parameter("mask1", [128, 256], F32, False)
    m2_d = nc.declare_dram_parameter("mask2", [NSTART, HALF], F32, False)
    ones_d = nc.declare_dram_parameter("ones", [1, 128], F32, False)
    eye32_d = nc.declare_dram_parameter("eye32", [NSTART, NSTART], F32, False)
    out_d = nc.declare_dram_parameter("out", [HALF, EMB], F32, True)

    NF = EMB // 128                  # 4 feature chunks
    NE = EMB // 128                  # 4 embed chunks
    NTI = TOT // 128 + 1             # 9 token tiles (last has 32 rows)
    rows_of = lambda ti: 128 if ti < NTI - 1 else TOT - 128 * (NTI - 1)

    sdt = BF16 if BF16_ATTN else F32R

    with tile.TileContext(nc) as tc:
        with (
            tc.tile_pool(name="const", bufs=1) as cpool,
            tc.tile_pool(name="big", bufs=1) as bpool,
            tc.tile_pool(name="work", bufs=3) as wpool,
            tc.tile_pool(name="ps", bufs=8, space="PSUM") as pspool,
        ):
            def psum(tag="ps"):
                return pspool.tile([128, 512], F32, tag=tag, name=tag, bufs=8)

            # ---- big inputs first: DMA triggers cost ~0.7us each and
            # serialize per engine, so the first K-phase operands must be
            # the first triggers on their queues -------------------------
            xt_flat = bpool.tile([128, NF * TOT], F32R, name="xt_flat")
            xt_sb = [xt_flat[:, fi * TOT:(fi + 1) * TOT] for fi in range(NF)]
            wkt_flat = cpool.tile([128, NF * EMB], F32R, name="wkt_flat")
            wkt_sb = [wkt_flat[:, ci * EMB:(ci + 1) * EMB] for ci in range(NF)]
            for fi in range(NF):
                nc.sync.dma_start(
                    wkt_sb[fi],
                    wkt_d[fi * 128:(fi + 1) * 128, :].bitcast(F32R))
                nc.scalar.dma_start(
                    xt_sb[fi],
                    xt_d[fi * 128:(fi + 1) * 128, :].bitcast(F32R))

            def load_w(name, dram, nchunk, eng):
                t_ = cpool.tile([128, nchunk * EMB], F32R, name=name)
                chunks = [t_[:, ci * EMB:(ci + 1) * EMB] for ci in range(nchunk)]
                for ci in range(nchunk):
                    eng.dma_start(
                        chunks[ci],
                        dram[ci * 128:(ci + 1) * 128, :].bitcast(F32R))
                return chunks

            wqt_sb = load_w("wqt_sb", wqt_d, NF, nc.sync)

            # small constants after the weight streams: cheap triggers,
            # needed only by the later DVE copy stages
            bkc_sb = cpool.tile([128, NE], F32, name="bkc_sb")
            nc.sync.dma_start(bkc_sb[:], bk_d[:])
            bqc_sb = cpool.tile([128, NE], F32, name="bqc_sb")
            nc.sync.dma_start(bqc_sb[:], bq_d[:])
            ones_sb = cpool.tile([1, 128], F32R, name="ones_sb")
            nc.sync.dma_start(ones_sb[:], ones_d[:].bitcast(F32R))
            bvr_sb = cpool.tile([1, EMB], F32R, name="bvr_sb")
            nc.sync.dma_start(bvr_sb[:], bv_d[:].bitcast(F32R))
            bur_sb = cpool.tile([1, EMB], F32R, name="bur_sb")
            nc.sync.dma_start(bur_sb[:], bu_d[:].bitcast(F32R))
            eye32_sb = cpool.tile([NSTART, NSTART], F32R, name="eye32_sb")
            nc.sync.dma_start(eye32_sb[:], eye32_d[:].bitcast(F32R))
            wvt_sb = load_w("wvt_sb", wvt_d, NF, nc.scalar)
            m1_sb = cpool.tile([128, 256], F32, name="m1_sb")
            nc.sync.dma_start(m1_sb[:], m1_d[:])
            m2_sb = cpool.tile([NSTART, HALF], F32, name="m2_sb")
            nc.sync.dma_start(m2_sb[:], m2_d[:])
            wut_sb = load_w("wut_sb", wut_d, 2 * EMB // 128, nc.sync)

            # ---- QKV projections ----------------------------------------
            # K^T only needed for own tokens; Q^T also for the 32 starts
            kq_spans = {"k": [(0, 512), (512, 512)],
                        "q": [(0, 512), (512, 512)]}
            kt_sb = [bpool.tile([128, HALF], sdt, name=f"kt_sb{ei}")
                     for ei in range(NE)]
            qt_sb = [bpool.tile([128, TOT], sdt, name=f"qt_sb{ei}")
                     for ei in range(NE)]
            # K first, fi-outer: the fi=0 matmuls only need the first
            # wkt/xt chunk DMAs, so PE starts ~2.5us in
            for t0, span in kq_spans["k"]:
                pss = [psum() for _ in range(NE)]
                for fi in range(NF):
                    for ei in range(NE):
                        nc.tensor.matmul(
                            pss[ei][:, :span],
                            wkt_sb[fi][:, ei * 128:(ei + 1) * 128],
                            xt_sb[fi][:, t0:t0 + span],
                            start=(fi == 0), stop=(fi == NF - 1))
                for ei in range(NE):
                    nc.vector.tensor_scalar_add(
                        kt_sb[ei][:, t0:t0 + span], pss[ei][:, :span],
                        bkc_sb[:, ei:ei + 1])

            # broadcast biases across partitions: bvb[p, e] = bv[e]
            # (emitted after the K matmuls so their late-arriving operand
            # DMAs don't block the head of the PE instruction stream)
            bvb_sb = cpool.tile([128, EMB], F32, name="bvb_sb")
            pb = psum()
            nc.tensor.matmul(pb[:, :EMB], ones_sb[:1, :], bvr_sb[:1, :],
                             start=True, stop=True)
            nc.vector.tensor_copy(bvb_sb[:], pb[:, :EMB])
            bub_sb = cpool.tile([128, EMB], F32, name="bub_sb")
            pb2 = psum()
            nc.tensor.matmul(pb2[:, :EMB], ones_sb[:1, :], bur_sb[:1, :],
                             start=True, stop=True)
            nc.vector.tensor_copy(bub_sb[:], pb2[:, :EMB])
            for ei in range(NE):
                for t0, span in kq_spans["q"]:
                    ps = psum()
                    for fi in range(NF):
                        nc.tensor.matmul(
                            ps[:, :span],
                            wqt_sb[fi][:, ei * 128:(ei + 1) * 128],
                            xt_sb[fi][:, t0:t0 + span],
                            start=(fi == 0), stop=(fi == NF - 1))
                    nc.vector.tensor_scalar_add(
                        qt_sb[ei][:, t0:t0 + span], ps[:, :span],
                        bqc_sb[:, ei:ei + 1])

            # Q tail (32 block-start tokens): project naturally with N=512
            # (stationary is just 32 columns -> cheap LDWEIGHTS), then
            # PE-transpose back to [e, 32]; bias lands in the copy
            qsn_ps = psum()
            for fi in range(NF):
                nc.tensor.matmul(qsn_ps[:NSTART, :],
                                 xt_sb[fi][:, HALF:TOT],
                                 wqt_sb[fi][:],
                                 start=(fi == 0), stop=(fi == NF - 1))
            qsn_sb = cpool.tile([NSTART, EMB], F32R, name="qsn_sb")
            nc.vector.tensor_copy(qsn_sb[:], qsn_ps[:NSTART, :])
            for ei in range(NE):
                tp = psum()
                nc.tensor.transpose(tp[:128, :NSTART].bitcast(F32R),
                                    qsn_sb[:, ei * 128:(ei + 1) * 128],
                                    eye32_sb[:, :])
                nc.vector.tensor_scalar_add(
                    qt_sb[ei][:, HALF:TOT], tp[:128, :NSTART],
                    bqc_sb[:, ei:ei + 1])

            vn_sb = [bpool.tile([128, EMB], sdt, name=f"vn_sb{ti}")
                     for ti in range(NTI)]
            for ti in range(NTI):
                r0, rows = ti * 128, rows_of(ti)
                ps = psum()
                for fi in range(NF):
                    nc.tensor.matmul(ps[:rows, :],
                                     xt_sb[fi][:, r0:r0 + rows],
                                     wvt_sb[fi][:],
                                     start=(fi == 0), stop=(fi == NF - 1))
                nc.vector.tensor_add(vn_sb[ti][:rows, :],
                                     ps[:rows, :], bvb_sb[:rows, :])
            av_v = vn_sb

            # ---- head2 ---------------------------------------------------
            s2m_sb = bpool.tile([NSTART, HALF], sdt, name="s2m_sb")
            for tt in range(2):
                t0 = tt * 512
                ps2 = psum()
                for ei in range(NE):
                    nc.tensor.matmul(ps2[:NSTART, :],
                                     qt_sb[ei][:, HALF:TOT],
                                     kt_sb[ei][:, t0:t0 + 512],
                                     start=(ei == 0), stop=(ei == NE - 1))
                nc.vector.tensor_mul(s2m_sb[:, t0:t0 + 512], ps2[:NSTART, :],
                                     m2_sb[:, t0:t0 + 512])

            h2t_sb = [bpool.tile([128, HALF], F32R, name=f"h2t_sb{ei}")
                      for ei in range(NE)]
            for ei in range(NE):
                for tt in range(2):
                    t0 = tt * 512
                    ph = psum()
                    nc.tensor.matmul(
                        ph[:, :],
                        av_v[NTI - 1][:NSTART, ei * 128:(ei + 1) * 128],
                        s2m_sb[:, t0:t0 + 512],
                        start=True, stop=True)
                    nc.scalar.copy(h2t_sb[ei][:, t0:t0 + 512], ph[:, :])

            # ---- head1 scores (all tiles first, decoupled from AV) ------
            # fp32r hits 1 cyc/row only at N>=256, so compute a 256-wide
            # strip of S^T and read just the valid left 128 columns.
            h1t_sb = [bpool.tile([128, HALF], F32R, name=f"h1t_sb{ei}")
                      for ei in range(NE)]
            s1ts = []
            for ti in range(HALF // 128):
                t0 = ti * 128
                sw = 128 if BF16_ATTN else min(256, HALF - t0)
                ps1 = psum()
                for ei in range(NE):
                    nc.tensor.matmul(ps1[:, :sw],
                                     qt_sb[ei][:, t0:t0 + 128],
                                     kt_sb[ei][:, t0:t0 + sw],
                                     start=(ei == 0), stop=(ei == NE - 1))
                s1t = wpool.tile([128, 256], sdt, tag="s1t", name="s1t",
                                 bufs=8)
                nc.vector.tensor_mul(s1t[:, :sw], ps1[:, :sw],
                                     m1_sb[:, :sw])
                s1ts.append((s1t, sw))

            # ---- head1 AV + output projection, interleaved per tile -----
            hcat = h1t_sb + h2t_sb
            for ti in range(HALF // 128):
                t0 = ti * 128
                s1t, sw = s1ts[ti]
                for ei in range(NE):
                    ph = psum()
                    nc.tensor.matmul(ph[:, :sw],
                                     av_v[ti][:, ei * 128:(ei + 1) * 128],
                                     s1t[:, :sw],
                                     start=True, stop=True)
                    nc.scalar.copy(h1t_sb[ei][:, t0:t0 + 128], ph[:, :128])
                po = psum()
                for ci in range(2 * EMB // 128):
                    nc.tensor.matmul(po[:, :],
                                     hcat[ci][:, t0:t0 + 128],
                                     wut_sb[ci],
                                     start=(ci == 0),
                                     stop=(ci == 2 * EMB // 128 - 1))
                ot = wpool.tile([128, EMB], F32, tag="ot", name="ot")
                nc.vector.tensor_add(ot[:], po[:, :], bub_sb[:])
                nc.scalar.dma_start(out_d[t0:t0 + 128, :], ot[:])

    return nc


_NC_CACHE = None


def _get_program():
    global _NC_CACHE
    if _NC_CACHE is None:
        nc = build_program()
        nc.compile()          # bacc passes: wait splitting, reg alloc, ISA
        _NC_CACHE = nc
    return _NC_CACHE


def _make_masks():
    tri = np.triu(np.ones((KBLK, KBLK), np.float32))           # [c_l, r_l]
    m1 = np.kron(np.eye(2, dtype=np.float32), tri)             # [128, 128]
    # mask2[h][i, rl] = 1 if 64*i <= h*HALF + rl
    r = np.arange(HALF)
    m2 = []
    for h in range(2):
        blk = (h * HALF + r) // KBLK                           # [HALF]
        m2.append((np.arange(NSTART)[:, None] <= blk[None, :])
                  .astype(np.float32))
    return m1, m2


def make_in_maps(inputs):
    x = np.asarray(inputs["x"], np.float32)
    wkt = np.ascontiguousarray(np.asarray(inputs["Wk"], np.float32).T)
    wqt = np.ascontiguousarray(np.asarray(inputs["Wq"], np.float32).T)
    wvt = np.ascontiguousarray(np.asarray(inputs["Wv"], np.float32).T)
    wut = np.ascontiguousarray(np.asarray(inputs["Wu"], np.float32).T)
    bk = np.asarray(inputs["bk"], np.float32)
    bq = np.asarray(inputs["bq"], np.float32)
    bv = np.asarray(inputs["bv"], np.float32)
    bu = np.asarray(inputs["bu"], np.float32)

    m1, m2 = _make_masks()
    m1w = np.concatenate([m1, np.zeros((128, 128), np.float32)], axis=1)
    starts = np.arange(NSTART) * KBLK

    in_maps = []
    for c in range(NCORES):
        b, h = c // 2, c % 2
        xin = np.concatenate(
            [x[b, h * HALF:(h + 1) * HALF], x[b, starts]], axis=0)
        in_maps.append({
            "xt": np.ascontiguousarray(xin.T),
            "wkt": wkt, "wqt": wqt, "wvt": wvt, "wut": wut,
            "bkc": np.ascontiguousarray(bk.reshape(EMB // 128, 128).T),
            "bqc": np.ascontiguousarray(bq.reshape(EMB // 128, 128).T),
            "bvr": bv.reshape(1, EMB).copy(),
            "bur": bu.reshape(1, EMB).copy(),
            "mask1": m1w, "mask2": m2[h],
            "ones": np.ones((1, 128), np.float32),
            "eye32": np.eye(NSTART, dtype=np.float32),
        })
    return in_maps


def _ensure_ntff_hook():
    """The agent image lacks antenv.axon_hooks; synthesize it and register
    the ctypes NTFF profiling hook so trace=True works under axon."""
    import importlib.util
    if importlib.util.find_spec("antenv.axon_hooks") is not None:
        return
    import types
    import antenv
    m = types.ModuleType("antenv.axon_hooks")
    m._hook = None
    def set_axon_ntff_profile_hook(h):
        m._hook = h
    def get_axon_ntff_profile_hook():
        return m._hook
    m.set_axon_ntff_profile_hook = set_axon_ntff_profile_hook
    m.get_axon_ntff_profile_hook = get_axon_ntff_profile_hook
    sys.modules["antenv.axon_hooks"] = m
    antenv.axon_hooks = m
    try:
        from trn_agent_boot.trn_boot import _ntff_profile_via_ctypes
        m._hook = _ntff_profile_via_ctypes("/opt/axon/libaxon_pjrt.so")
    except Exception:
        pass


def run_sharded(inputs, trace=False, trace_kwargs=None):
    """inputs: dict of full numpy arrays keyed like setup_inputs().
    Returns (full_output [B, T, EMB] float32, BassKernelResults)."""
    if trace:
        _ensure_ntff_hook()
    in_maps = make_in_maps(inputs)
    nc = _get_program()
    res = run_bass_kernel_spmd(nc, in_maps, list(range(NCORES)), trace=trace,
                               **(trace_kwargs or {}))

    out = np.empty((B, T, EMB), np.float32)
    for c in range(NCORES):
        b, h = c // 2, c % 2
        out[b, h * HALF:(h + 1) * HALF] = res.results[c]["out"]
    return out, res


def kernel(**inputs):
    out, _ = run_sharded(inputs, trace=False)
    return out



# revision 3
# speedup vs baseline: 1.5657x; 1.5657x over previous
"""Trainium2 Bass kernel for nn_BlocksparseFixedSelfAttention.

Reference computation (B=4, T=2048, EMB=512, KBLK=64):
    Kt = x @ Wk.T + bk ; Qt = x @ Wq.T + bq ; Vt = x @ Wv.T + bv
    head1: block-causal local attention inside each 64-token block
           (row j attends cols [block_start(j) .. j], S = K Q^T)
    head2: row r attends every block start c = 64*i with c <= r
    out = concat(h1, h2) @ Wu.T + bu

Key algebraic restructure (v2): the output projection is folded into V.
With Wu = [Wu1 | Wu2] (columns 0:512 / 512:1024):
    out = h1 @ Wu1^T + h2 @ Wu2^T + bu
        = sum_blk tril(K_b Q_b^T) (V_b @ Wu1^T)  +  S2 (V_s @ Wu2^T) + bu
so the device computes V1 = x @ (Wv^T Wu1^T) for own tokens and
V2s = x_starts @ (Wv^T Wu2^T) for the 32 block starts; the two AV
matmuls then accumulate directly into the same [128, 512] PSUM tile and
the result is stored with no output GEMM at all. The weight products
are precomputed on the host (pure weight preprocessing).

All matmul operands are bf16 (host-converted), f32 PSUM accumulate:
1 cyc/row on PE at any moving size, and half the HBM traffic.
Measured numerically: rel err ~5e-3 vs the f32 reference (tol 2e-2).

Sharding: data-parallel over (batch, T-half) -> 8 shards, one per core.
Each core gets its 1024 own token rows of x plus the 32 block-start
rows, feature-major (x^T), replicated (pre-folded) weights, and
produces its [1024, 512] slice of the output.
"""

import os
import sys

import numpy as np

for _p in ("/opt/trn_rl_repo",):
    if _p not in sys.path and os.path.isdir(_p):
        sys.path.append(_p)

import ml_dtypes

from concourse import bass, bacc, mybir
from concourse import tile
from concourse.bass_utils import run_bass_kernel_spmd

T = 2048
KBLK = 64
EMB = 512
B = 4
NCORES = 8
HALF = T // 2            # tokens owned per core
NSTART = T // KBLK       # 32 block starts
TOT = HALF + NSTART      # own tokens + appended block-start tokens
F32 = mybir.dt.float32
F32R = mybir.dt.float32r
BF16 = mybir.dt.bfloat16
BF16NP = ml_dtypes.bfloat16

NF = EMB // 128          # 4 feature chunks (contraction)
NE = EMB // 128          # 4 embed chunks
NTI = HALF // 128        # 8 own-token tiles


def build_program(with_bias):
    nc = bacc.Bacc("TRN2", target_bir_lowering=False, debug=False)

    xt_d = nc.declare_dram_parameter("xt", [EMB, TOT], BF16, False)
    wkt_d = nc.declare_dram_parameter("wkt", [EMB, EMB], BF16, False)
    wqt_d = nc.declare_dram_parameter("wqt", [EMB, EMB], BF16, False)
    wvu1_d = nc.declare_dram_parameter("wvu1", [EMB, EMB], BF16, False)
    wvu2_d = nc.declare_dram_parameter("wvu2", [EMB, EMB], BF16, False)
    m1_d = nc.declare_dram_parameter("mask1", [128, 128], F32, False)
    m2_d = nc.declare_dram_parameter("mask2", [NSTART, HALF], F32, False)
    bk_d = nc.declare_dram_parameter("bkc", [128, NE], F32, False)
    bq_d = nc.declare_dram_parameter("bqc", [128, NE], F32, False)
    if with_bias:
        bv1_d = nc.declare_dram_parameter("bv1r", [1, EMB], F32, False)
        bv2_d = nc.declare_dram_parameter("bv2r", [1, EMB], F32, False)
        bu_d = nc.declare_dram_parameter("bur", [1, EMB], F32, False)
        ones_d = nc.declare_dram_parameter("ones", [1, 128], F32, False)
    out_d = nc.declare_dram_parameter("out", [HALF, EMB], F32, True)

    with tile.TileContext(nc) as tc:
        with (
            tc.tile_pool(name="const", bufs=1) as cpool,
            tc.tile_pool(name="big", bufs=1) as bpool,
            tc.tile_pool(name="work", bufs=3) as wpool,
            tc.tile_pool(name="ps", bufs=8, space="PSUM") as pspool,
        ):
            def psum(tag="ps"):
                return pspool.tile([128, 512], F32, tag=tag, name=tag, bufs=8)

            # ---- DMA: the K-phase operands (wkt, xt) must be the first
            # triggers on their queues so the PE can start ~2.5us in ------
            wkt_flat = cpool.tile([128, NF * EMB], BF16, name="wkt_flat")
            wkt_sb = [wkt_flat[:, fi * EMB:(fi + 1) * EMB] for fi in range(NF)]
            xt_flat = bpool.tile([128, NF * TOT], BF16, name="xt_flat")
            xt_sb = [xt_flat[:, fi * TOT:(fi + 1) * TOT] for fi in range(NF)]
            for fi in range(NF):
                nc.sync.dma_start(wkt_sb[fi], wkt_d[fi * 128:(fi + 1) * 128, :])
                nc.scalar.dma_start(xt_sb[fi], xt_d[fi * 128:(fi + 1) * 128, :])

            def load_w(name, dram, eng):
                t_ = cpool.tile([128, NF * EMB], BF16, name=name)
                chunks = [t_[:, ci * EMB:(ci + 1) * EMB] for ci in range(NF)]
                for ci in range(NF):
                    eng.dma_start(chunks[ci], dram[ci * 128:(ci + 1) * 128, :])
                return chunks

            wqt_sb = load_w("wqt_sb", wqt_d, nc.sync)
            bkc_sb = cpool.tile([128, NE], F32, name="bkc_sb")
            nc.sync.dma_start(bkc_sb[:], bk_d[:])
            bqc_sb = cpool.tile([128, NE], F32, name="bqc_sb")
            nc.sync.dma_start(bqc_sb[:], bq_d[:])
            m1_sb = cpool.tile([128, 128], F32, name="m1_sb")
            nc.sync.dma_start(m1_sb[:], m1_d[:])
            m2_sb = cpool.tile([NSTART, HALF], F32, name="m2_sb")
            nc.sync.dma_start(m2_sb[:], m2_d[:])
            wvu1_sb = load_w("wvu1_sb", wvu1_d, nc.scalar)
            wvu2_sb = load_w("wvu2_sb", wvu2_d, nc.scalar)
            if with_bias:
                ones_sb = cpool.tile([1, 128], F32R, name="ones_sb")
                nc.sync.dma_start(ones_sb[:], ones_d[:].bitcast(F32R))
                bv1r_sb = cpool.tile([1, EMB], F32R, name="bv1r_sb")
                nc.sync.dma_start(bv1r_sb[:], bv1_d[:].bitcast(F32R))
                bv2r_sb = cpool.tile([1, EMB], F32R, name="bv2r_sb")
                nc.sync.dma_start(bv2r_sb[:], bv2_d[:].bitcast(F32R))
                bur_sb = cpool.tile([1, EMB], F32R, name="bur_sb")
                nc.sync.dma_start(bur_sb[:], bu_d[:].bitcast(F32R))

            # ---- K^T projection (own tokens), fi-outer so the first
            # matmuls only need the first wkt/xt chunk DMAs ---------------
            kt_sb = [bpool.tile([128, HALF], BF16, name=f"kt_sb{ei}")
                     for ei in range(NE)]
            for t0 in (0, 512):
                pss = [psum() for _ in range(NE)]
                for fi in range(NF):
                    for ei in range(NE):
                        nc.tensor.matmul(
                            pss[ei][:, :512],
                            wkt_sb[fi][:, ei * 128:(ei + 1) * 128],
                            xt_sb[fi][:, t0:t0 + 512],
                            start=(fi == 0), stop=(fi == NF - 1))
                for ei in range(NE):
                    nc.vector.tensor_scalar_add(
                        kt_sb[ei][:, t0:t0 + 512], pss[ei][:, :512],
                        bkc_sb[:, ei:ei + 1])

            # broadcast row-biases across partitions via rank-1 matmuls
            if with_bias:
                bcast = {}
                for nm, src in (("bv1", bv1r_sb), ("bv2", bv2r_sb),
                                ("bu", bur_sb)):
                    pb = psum()
                    nc.tensor.matmul(pb[:, :EMB], ones_sb[:1, :], src[:1, :],
                                     start=True, stop=True)
                    bb = cpool.tile([128, EMB], F32, name=f"{nm}b_sb")
                    nc.vector.tensor_copy(bb[:], pb[:, :EMB])
                    bcast[nm] = bb

            # ---- Q^T projection (own tokens + 32 starts) ----------------
            # (ei, fi) outer with the three t-spans inside: consecutive
            # matmuls share one stationary, so the 32-wide tail's LDWEIGHTS
            # hides under the preceding 512-row matmul.
            qt_sb = [bpool.tile([128, TOT], BF16, name=f"qt_sb{ei}")
                     for ei in range(NE)]
            spans = [(0, 512), (512, 512), (1024, NSTART)]
            for ei in range(NE):
                pss = [psum() for _ in spans]
                for fi in range(NF):
                    for si, (t0, sw) in enumerate(spans):
                        nc.tensor.matmul(
                            pss[si][:, :sw],
                            wqt_sb[fi][:, ei * 128:(ei + 1) * 128],
                            xt_sb[fi][:, t0:t0 + sw],
                            start=(fi == 0), stop=(fi == NF - 1))
                for si, (t0, sw) in enumerate(spans):
                    nc.vector.tensor_scalar_add(
                        qt_sb[ei][:, t0:t0 + sw], pss[si][:, :sw],
                        bqc_sb[:, ei:ei + 1])

            # ---- V2s = x_starts @ (Wv^T Wu2^T) --------------------------
            ps = psum()
            for fi in range(NF):
                nc.tensor.matmul(ps[:NSTART, :], xt_sb[fi][:, HALF:TOT],
                                 wvu2_sb[fi][:],
                                 start=(fi == 0), stop=(fi == NF - 1))
            v2s_sb = cpool.tile([NSTART, EMB], BF16, name="v2s_sb")
            if with_bias:
                nc.vector.tensor_add(v2s_sb[:], ps[:NSTART, :],
                                     bcast["bv2"][:NSTART, :])
            else:
                nc.scalar.copy(v2s_sb[:], ps[:NSTART, :])

            # ---- head2 scores S2^T[i, r], masked ------------------------
            s2t_sb = bpool.tile([NSTART, HALF], BF16, name="s2t_sb")
            for t0 in (0, 512):
                ps2 = psum()
                for ei in range(NE):
                    nc.tensor.matmul(ps2[:NSTART, :512],
                                     qt_sb[ei][:, HALF:TOT],
                                     kt_sb[ei][:, t0:t0 + 512],
                                     start=(ei == 0), stop=(ei == NE - 1))
                nc.vector.tensor_mul(s2t_sb[:, t0:t0 + 512], ps2[:NSTART, :512],
                                     m2_sb[:, t0:t0 + 512])

            # ---- main tile pipeline: V1 projection + head1 scores for
            # tile ti, then AV + store for tile ti-1. The small score/AV
            # LDWEIGHTS hide under the 512-row V1 matmuls, and the DVE
            # mask-multiply of tile ti overlaps the AV of ti-1 -------------
            v1n_sb = [bpool.tile([128, EMB], BF16, name=f"v1n_sb{ti}")
                      for ti in range(NTI)]
            s1ts = [None] * NTI

            def emit_v1_s1(ti):
                t0 = ti * 128
                ps = psum()
                for fi in range(NF):
                    nc.tensor.matmul(ps[:, :], xt_sb[fi][:, t0:t0 + 128],
                                     wvu1_sb[fi][:],
                                     start=(fi == 0), stop=(fi == NF - 1))
                if with_bias:
                    nc.vector.tensor_add(v1n_sb[ti][:], ps[:, :],
                                         bcast["bv1"][:])
                else:
                    nc.scalar.copy(v1n_sb[ti][:], ps[:, :])
                ps1 = psum()
                for ei in range(NE):
                    nc.tensor.matmul(ps1[:, :128],
                                     qt_sb[ei][:, t0:t0 + 128],
                                     kt_sb[ei][:, t0:t0 + 128],
                                     start=(ei == 0), stop=(ei == NE - 1))
                s1t = wpool.tile([128, 128], BF16, tag="s1t", name="s1t",
                                 bufs=4)
                nc.vector.tensor_mul(s1t[:], ps1[:, :128], m1_sb[:])
                s1ts[ti] = s1t

            def emit_av_out(ti):
                t0 = ti * 128
                ph = psum()
                nc.tensor.matmul(ph[:, :], s1ts[ti][:], v1n_sb[ti][:],
                                 start=True, stop=False)
                nc.tensor.matmul(ph[:, :], s2t_sb[:, t0:t0 + 128],
                                 v2s_sb[:], start=False, stop=True)
                ot = wpool.tile([128, EMB], F32, tag="ot", name="ot", bufs=3)
                if with_bias:
                    nc.vector.tensor_add(ot[:], ph[:, :], bcast["bu"][:])
                else:
                    nc.scalar.copy(ot[:], ph[:, :])
                nc.sync.dma_start(out_d[t0:t0 + 128, :], ot[:])

            for ti in range(NTI + 1):
                if ti < NTI:
                    emit_v1_s1(ti)
                if ti >= 1:
                    emit_av_out(ti - 1)

    return nc


_NC_CACHE = {}


def _get_program(with_bias):
    if with_bias not in _NC_CACHE:
        nc = build_program(with_bias)
        nc.compile()          # bacc passes: wait splitting, reg alloc, ISA
        _NC_CACHE[with_bias] = nc
    return _NC_CACHE[with_bias]


def _make_masks():
    tri = np.triu(np.ones((KBLK, KBLK), np.float32))           # [c_l, r_l]
    m1 = np.kron(np.eye(2, dtype=np.float32), tri)             # [128, 128]
    # mask2[h][i, rl] = 1 if 64*i <= h*HALF + rl
    r = np.arange(HALF)
    m2 = []
    for h in range(2):
        blk = (h * HALF + r) // KBLK                           # [HALF]
        m2.append((np.arange(NSTART)[:, None] <= blk[None, :])
                  .astype(np.float32))
    return m1, m2


def make_in_maps(inputs, with_bias):
    x = np.asarray(inputs["x"], np.float32)
    wk = np.asarray(inputs["Wk"], np.float32)
    wq = np.asarray(inputs["Wq"], np.float32)
    wv = np.asarray(inputs["Wv"], np.float32)
    wu = np.asarray(inputs["Wu"], np.float32)
    bk = np.asarray(inputs["bk"], np.float32)
    bq = np.asarray(inputs["bq"], np.float32)
    bv = np.asarray(inputs["bv"], np.float32)
    bu = np.asarray(inputs["bu"], np.float32)

    wkt = np.ascontiguousarray(wk.T).astype(BF16NP)
    wqt = np.ascontiguousarray(wq.T).astype(BF16NP)
    # fold the output projection into V (host-side weight preprocessing)
    wvu1 = np.ascontiguousarray(wv.T @ wu[:, :EMB].T).astype(BF16NP)
    wvu2 = np.ascontiguousarray(wv.T @ wu[:, EMB:].T).astype(BF16NP)
    bv1 = (bv @ wu[:, :EMB].T).reshape(1, EMB).copy()
    bv2 = (bv @ wu[:, EMB:].T).reshape(1, EMB).copy()

    m1, m2 = _make_masks()
    starts = np.arange(NSTART) * KBLK

    in_maps = []
    for c in range(NCORES):
        b, h = c // 2, c % 2
        xin = np.concatenate(
            [x[b, h * HALF:(h + 1) * HALF], x[b, starts]], axis=0)
        m = {
            "xt": np.ascontiguousarray(xin.T.astype(BF16NP)),
            "wkt": wkt, "wqt": wqt, "wvu1": wvu1, "wvu2": wvu2,
            "mask1": m1, "mask2": m2[h],
            "bkc": np.ascontiguousarray(bk.reshape(EMB // 128, 128).T),
            "bqc": np.ascontiguousarray(bq.reshape(EMB // 128, 128).T),
        }
        if with_bias:
            m.update({
                "bv1r": bv1, "bv2r": bv2, "bur": bu.reshape(1, EMB).copy(),
                "ones": np.ones((1, 128), np.float32),
            })
        in_maps.append(m)
    return in_maps


def _ensure_ntff_hook():
    """The agent image lacks antenv.axon_hooks; synthesize it and register
    the ctypes NTFF profiling hook so trace=True works under axon."""
    import importlib.util
    if importlib.util.find_spec("antenv.axon_hooks") is not None:
        return
    import types
    import antenv
    m = types.ModuleType("antenv.axon_hooks")
    m._hook = None
    def set_axon_ntff_profile_hook(h):
        m._hook = h
    def get_axon_ntff_profile_hook():
        return m._hook
    m.set_axon_ntff_profile_hook = set_axon_ntff_profile_hook
    m.get_axon_ntff_profile_hook = get_axon_ntff_profile_hook
    sys.modules["antenv.axon_hooks"] = m
    antenv.axon_hooks = m
    try:
        from trn_agent_boot.trn_boot import _ntff_profile_via_ctypes
        m._hook = _ntff_profile_via_ctypes("/opt/axon/libaxon_pjrt.so")
    except Exception:
        pass


def run_sharded(inputs, trace=False, trace_kwargs=None):
    """inputs: dict of full numpy arrays keyed like setup_inputs().
    Returns (full_output [B, T, EMB] float32, BassKernelResults)."""
    if trace:
        _ensure_ntff_hook()
    with_bias = any(
        float(np.abs(np.asarray(inputs[k])).max()) != 0.0
        for k in ("bk", "bq", "bv", "bu"))
    in_maps = make_in_maps(inputs, with_bias)
    nc = _get_program(with_bias)
    res = run_bass_kernel_spmd(nc, in_maps, list(range(NCORES)), trace=trace,
                               **(trace_kwargs or {}))

    out = np.empty((B, T, EMB), np.float32)
    for c in range(NCORES):
        b, h = c // 2, c % 2
        out[b, h * HALF:(h + 1) * HALF] = res.results[c]["out"]
    return out, res


def kernel(**inputs):
    out, _ = run_sharded(inputs, trace=False)
    return out
